# revision 16
# baseline (speedup 1.0000x reference)
"""Trainium2 Bass kernel for nn_ClassConditionalAffinity.

Problem (hardcoded shapes): B=4, D=256, H=W=64, grid=16 -> HW=4096.
Valid pairs are the 4-neighbors of the 16x16 grid of pixels (0,4,...,60)^2
(manhattan distance 4 <= 5), giving 960 directed pairs per batch. The
output A is (B, 4096, 4096): identity everywhere except the 256 grid rows,
which carry up to 4 sigmoid(MLP) affinities at columns row+-4 / row+-256,
then every row is normalized by its sum.

Sharding: 8 cores = 4 batches x 2 row-halves (2048 rows each). Every core
runs the SAME program; per-core differences are carried by the data:
  - features/embeddings are passed as a 10-grid-row halo window (zero
    padded at the outer boundary) and boundary masks zero the missing
    north/south neighbor values,
  - the upper-half cores write columns rotated by -2048 (mod 4096); the
    host un-rotates with np.roll. Every DMA offset is a compile-time
    constant shared by all 8 cores.

The kernel is HBM-write-bound: 2048x4096 fp16 = 16.8 MB/core (fp16 halves
the f32 traffic; harness tolerance is 2e-2, fp16 adds ~5e-4; the host
casts back to f32 on gather). Output strategy (sync HWDGE ring, in FIFO
order -- each SDMA engine owns fixed partitions and drains its ring in
order, so later DMAs to the same bytes land later):
  1. ONE 16.8 MB full-shard zero DMA whose source is a tiny [128, 128]
     zero tile read through a stride-0 broadcast access pattern.
  2. ONE merged DMA overwriting the 8 odd-block identity diagonals (a
     [128,128] fp16 identity broadcast across blocks; the flat-DRAM
     k-stride 256*4096+256 lands each copy on the block diagonal).
  3. After the MLP: 3 merged DMAs (plus 4 small ones for the wrapping
     block 0) overwriting only the NONZERO patch columns of the 8 even
     blocks: the -256 diagonal, the 136-wide -4/0/+4 band, the +256
     diagonal. Zero flanks are already covered by (1).
The MLP (fp16 weights/activations, f32 PSUM + f32 sums/reciprocal) runs
entirely under the zero-write drain. The tiny V-table scatter goes via
gpsimd (SWDGE) so it never queues behind bulk HWDGE traffic; input loads
go on the scalar HWDGE ring.
"""

import os
import numpy as np

import concourse.bass as bass
import concourse.mybir as mybir
import concourse.tile as tile
from concourse import bacc
from concourse.bass import broadcast_tensor_aps
from concourse.bass_utils import run_bass_kernel_spmd
from concourse.masks import make_identity

F32 = mybir.dt.float32
F16 = mybir.dt.float16
AF = mybir.ActivationFunctionType

B, D, H, W = 4, 256, 64, 64
HW = H * W                      # 4096
G = 16                          # grid points per axis
TG = 8                          # own grid rows (gi) per core
ROWS = 2048                     # rows per core shard
NB = 16                         # 128-row blocks per shard
NPAIR = 496                     # E/W: 8*15 each, N/S: 8*16 each
MPAD = 512
MLP_IN, H1, H2 = 640, 256, 128
KSTR = 256 * HW + 256           # flat stride between consecutive diag/patch blocks

LAST_RESULTS = None             # test.py reads exec_time_ns from here


def _build_nc():
    nc = bacc.Bacc("TRN2", target_bir_lowering=False)

    feat = nc.dram_tensor("feat", [D, 10, G], F16, kind="ExternalInput")
    embt = nc.dram_tensor("embt", [128, 10 * G], F16, kind="ExternalInput")
    w1 = nc.dram_tensor("w1", [MLP_IN, H1], F16, kind="ExternalInput")
    w2 = nc.dram_tensor("w2", [H1, H2], F16, kind="ExternalInput")
    w3 = nc.dram_tensor("w3", [H2, 1], F16, kind="ExternalInput")
    misc = nc.dram_tensor("misc", [128, 4], F32, kind="ExternalInput")
    mns = nc.dram_tensor("mns", [1, 256], F32, kind="ExternalInput")
    a = nc.dram_tensor("a", [ROWS, HW], F16, kind="ExternalOutput")

    from contextlib import ExitStack

    with tile.TileContext(nc) as tc, ExitStack() as ctx:
        consts = ctx.enter_context(tc.tile_pool(name="consts", bufs=1))
        dpool = ctx.enter_context(tc.tile_pool(name="dpool", bufs=2))
        psum = ctx.enter_context(tc.tile_pool(name="psum", bufs=1, space="PSUM"))

        aflat = a[:].rearrange("r c -> (r c)")

        def dram_ap(offset, dims):
            return bass.AP(aflat.tensor, offset, dims)

        # ---- tiny constants ----
        # zsrc memset is split DVE/gpsimd so the first bulk write can start
        # ~2 us earlier than a single-engine memset would allow
        zsrc = consts.tile([128, HW], F16)
        nc.vector.memset(zsrc[:, 0:2048], 0.0)
        nc.gpsimd.memset(zsrc[:, 2048:HW], 0.0)

        # ---- (1) odd-block zeros: ONE merged 8.4 MB DMA (stride-0 src) ----
        out_zero = dram_ap(128 * HW, [[HW, 128], [2 * 128 * HW, TG], [1, HW]])
        in_zero = zsrc[:].rearrange("p (j c) -> p j c", j=1)
        bi, bo = broadcast_tensor_aps(in_zero, out_zero)
        nc.sync.dma_start(out=bo, in_=bi)

        # ---- (2) odd-block identity diagonals: one merged DMA, same ring
        # as (1) so its descriptors land after the zeros on every engine ----
        identh = consts.tile([128, 128], F16)
        make_identity(nc, identh)
        out_diag = dram_ap(128 * HW + 128, [[HW, 128], [KSTR, TG], [1, 128]])
        in_diag = identh[:].rearrange("p (j c) -> p j c", j=1)
        bi, bo = broadcast_tensor_aps(in_diag, out_diag)
        nc.sync.dma_start(out=bo, in_=bi)

        # ---- (3) even-block zero stripes around the 640-wide patch
        # windows (disjoint from the patches -> no ordering constraint) ----
        nc.sync.dma_start(out=a[0:128, 384:3840], in_=zsrc[:, 0:3456])
        for k in range(1, TG):
            r0 = 256 * k
            c0 = 256 * k - 256
            if c0 > 0:
                nc.sync.dma_start(out=a[r0 : r0 + 128, 0:c0], in_=zsrc[:, 0:c0])
            nc.sync.dma_start(
                out=a[r0 : r0 + 128, c0 + 640 : HW], in_=zsrc[:, 0 : HW - c0 - 640]
            )

        # ---- inputs (scalar HWDGE ring) ----
        g0 = consts.tile([128, 10, G], F16)
        g1 = consts.tile([128, 10, G], F16)
        nc.scalar.dma_start(out=g0, in_=feat[0:128])
        nc.scalar.dma_start(out=g1, in_=feat[128:256])
        emb = consts.tile([128, 10, G], F16)
        nc.scalar.dma_start(out=emb.rearrange("p t g -> p (t g)"), in_=embt[:])
        w1sb = consts.tile([128, 5, H1], F16)
        nc.scalar.dma_start(out=w1sb, in_=w1.rearrange("(k p) n -> p k n", p=128))
        w2sb = consts.tile([128, 2, H2], F16)
        nc.scalar.dma_start(out=w2sb, in_=w2.rearrange("(k p) n -> p k n", p=128))
        w3sb = consts.tile([128, 1], F16)
        nc.scalar.dma_start(out=w3sb, in_=w3[:])
        miscs = consts.tile([128, 4], F32)
        nc.scalar.dma_start(out=miscs, in_=misc[:])
        mnssb = consts.tile([1, 256], F32)
        nc.scalar.dma_start(out=mnssb, in_=mns[:])
        mn = mnssb[:, 0:128]
        ms = mnssb[:, 128:256]

        # ---- assemble xT (640 x 512) fp16, pair order: E | W | N | S ----
        # pair classes, local own gi index t=0..7 lives at halo row t+1
        xt = [consts.tile([128, MPAD], F16, name=f"xt{k}") for k in range(5)]
        for k in range(5):
            nc.vector.memset(xt[k][:, NPAIR:MPAD], 0.0)

        # pair storage is (g, t)-major: idx = g*8 + t (t contiguous)
        def cview(apx, lo, n):
            return apx[:, lo : lo + n].rearrange("p (g t) -> p g t", t=TG)

        def gswap(apx):
            return apx.rearrange("p t g -> p g t")

        for ki, gt in ((0, g0), (1, g1)):
            f1a, f2a = xt[ki], xt[ki + 2]
            # E: f1=(t,0:15) f2=(t,1:16)
            nc.vector.tensor_copy(cview(f1a, 0, 120), gswap(gt[:, 1:9, 0:15]))
            nc.vector.tensor_copy(cview(f2a, 0, 120), gswap(gt[:, 1:9, 1:16]))
            # W: f1=(t,1:16) f2=(t,0:15)
            nc.vector.tensor_copy(cview(f1a, 120, 120), gswap(gt[:, 1:9, 1:16]))
            nc.vector.tensor_copy(cview(f2a, 120, 120), gswap(gt[:, 1:9, 0:15]))
            # N: f1=own rows, f2=rows above (halo index t)
            nc.vector.tensor_copy(cview(f1a, 240, 128), gswap(gt[:, 1:9, :]))
            nc.vector.tensor_copy(cview(f2a, 240, 128), gswap(gt[:, 0:8, :]))
            # S: f2=rows below (halo index t+2)
            nc.vector.tensor_copy(cview(f1a, 368, 128), gswap(gt[:, 1:9, :]))
            nc.vector.tensor_copy(cview(f2a, 368, 128), gswap(gt[:, 2:10, :]))
        # coord rows: 0.5*(emb[p1]+emb[p2]) with the 0.5 folded in on host
        ct = xt[4]
        nc.vector.tensor_add(cview(ct, 0, 120), gswap(emb[:, 1:9, 0:15]), gswap(emb[:, 1:9, 1:16]))
        nc.vector.tensor_add(cview(ct, 120, 120), gswap(emb[:, 1:9, 1:16]), gswap(emb[:, 1:9, 0:15]))
        nc.vector.tensor_add(cview(ct, 240, 128), gswap(emb[:, 1:9, :]), gswap(emb[:, 0:8, :]))
        nc.vector.tensor_add(cview(ct, 368, 128), gswap(emb[:, 1:9, :]), gswap(emb[:, 2:10, :]))

        # ---- MLP (fp16 in, f32 PSUM, transposed activations) ----
        h1sb = consts.tile([128, 2, MPAD], F16)
        for n in range(2):
            ps1 = psum.tile([128, MPAD], F32)
            for k in range(5):
                nc.tensor.matmul(
                    ps1,
                    w1sb[:, k, 128 * n : 128 * (n + 1)],
                    xt[k][:],
                    start=(k == 0),
                    stop=(k == 4),
                )
            nc.scalar.activation(h1sb[:, n, :], ps1, AF.Relu, bias=miscs[:, n : n + 1])
        ps2 = psum.tile([128, MPAD], F32)
        for k in range(2):
            nc.tensor.matmul(ps2, w2sb[:, k, :], h1sb[:, k, :], start=(k == 0), stop=(k == 1))
        h2sb = consts.tile([128, MPAD], F16)
        nc.scalar.activation(h2sb, ps2, AF.Relu, bias=miscs[:, 2:3])
        ps3 = psum.tile([1, MPAD], F32)
        nc.tensor.matmul(ps3, w3sb[:], h2sb[:], start=True, stop=True)
        vals = consts.tile([1, MPAD], F32)
        nc.scalar.activation(vals, ps3, AF.Sigmoid, bias=miscs[0:1, 3:4])

        # ---- row sums (f32), reciprocal, scaled values -> vall fp16 ----
        vnm = consts.tile([1, 128], F32)
        vsm = consts.tile([1, 128], F32)
        nc.vector.tensor_mul(vnm, vals[:, 240:368], mn)
        nc.vector.tensor_mul(vsm, vals[:, 368:496], ms)

        s = consts.tile([1, 128], F32)
        nc.vector.memset(s, 1.0)
        s3 = s.rearrange("o (g t) -> o g t", t=TG)
        nc.vector.tensor_add(s3[:, 0:15, :], s3[:, 0:15, :], cview(vals, 0, 120))
        nc.vector.tensor_add(s3[:, 1:16, :], s3[:, 1:16, :], cview(vals, 120, 120))
        nc.vector.tensor_add(s, s, vnm[:])
        nc.vector.tensor_add(s, s, vsm[:])
        recip = consts.tile([1, 128], F32)
        nc.vector.reciprocal(recip, s)
        r3 = recip.rearrange("o (g t) -> o g t", t=TG)

        # vall layout (g, k, t); offsets k: 0:-256(N) 1:-4(W) 2:diag 3:+4(E) 4:+256(S)
        vall = consts.tile([1, 16 * 5 * TG], F16)
        nc.vector.memset(vall, 0.0)
        va4 = vall.rearrange("o (g k t) -> o g k t", k=5, t=TG)
        nc.vector.tensor_copy(va4[:, :, 2, :], r3)
        nc.vector.tensor_mul(va4[:, :, 0, :], vnm.rearrange("o (g t) -> o g t", t=TG), r3)
        nc.vector.tensor_mul(va4[:, :, 4, :], vsm.rearrange("o (g t) -> o g t", t=TG), r3)
        nc.vector.tensor_mul(va4[:, 0:15, 3, :], cview(vals, 0, 120), r3[:, 0:15, :])
        nc.vector.tensor_mul(va4[:, 1:16, 1, :], cview(vals, 120, 120), r3[:, 1:16, :])

        # ---- V table (fp16): SWDGE scatter, partition 4g gets 40 values ----
        v = consts.tile([128, 5, TG], F16)
        nc.gpsimd.memset(v, 0.0)
        nc.gpsimd.memset(v[:, 2, :], 1.0)
        with nc.allow_non_contiguous_dma(reason="tiny per-partition scatter"):
            nc.gpsimd.dma_start(
                out=v[0:61:4, :, :],
                in_=vall.rearrange("o (g f) -> o g f", g=16),
            )

        # ---- batched patch build (all 8 blocks at once, fp16, 640-wide:
        # 1280 B lines stay above the 512 B HBM read-modify-write knee) ----
        def vb(k):  # v[:, k, :] broadcast over the diag columns
            return v[:, k, :].rearrange("p (t c) -> p t c", c=1)

        def idb():  # identity broadcast over the 8 blocks
            return identh[:].rearrange("p (j c) -> p j c", j=1)

        ph4 = consts.tile([128, TG, 640], F16)
        nc.vector.memset(ph4[:, :, 128:512], 0.0)
        bi0, bi1 = broadcast_tensor_aps(idb(), vb(0))
        nc.vector.tensor_mul(ph4[:, :, 0:128], bi0, bi1)
        bi0, bi1 = broadcast_tensor_aps(idb(), vb(4))
        nc.vector.tensor_mul(ph4[:, :, 512:640], bi0, bi1)
        bi0, bi1 = broadcast_tensor_aps(idb(), vb(1))
        nc.vector.tensor_mul(ph4[:, :, 252:380], bi0, bi1)
        dt1 = dpool.tile([128, TG, 128], F16)
        bi0, bi1 = broadcast_tensor_aps(idb(), vb(2))
        nc.vector.tensor_mul(dt1, bi0, bi1)
        nc.vector.tensor_add(ph4[:, :, 256:384], ph4[:, :, 256:384], dt1[:])
        dt2 = dpool.tile([128, TG, 128], F16)
        bi0, bi1 = broadcast_tensor_aps(idb(), vb(3))
        nc.vector.tensor_mul(dt2, bi0, bi1)
        nc.vector.tensor_add(ph4[:, :, 260:388], ph4[:, :, 260:388], dt2[:])

        # ---- patch windows (scalar ring; disjoint from all zero writes,
        # so they drain as soon as ph4 is ready) ----
        # wrap block lb=0: window starts at col -256 (mod 4096)
        nc.scalar.dma_start(out=a[0:128, 3840:4096], in_=ph4[:, 0, 0:256])
        nc.scalar.dma_start(out=a[0:128, 0:384], in_=ph4[:, 0, 256:640])
        # blocks lb=2k, k=1..7: window at col 256k-256
        nc.scalar.dma_start(
            out=dram_ap(256 * HW, [[HW, 128], [KSTR, 7], [1, 640]]),
            in_=ph4[:, 1:8, :],
        )
    nc.compile()  # bacc register allocation — required before NEFF compile
    return nc


_NC_CACHE = None


def _get_nc():
    global _NC_CACHE
    if _NC_CACHE is None:
        _NC_CACHE = _build_nc()
    return _NC_CACHE


def kernel(**inputs) -> np.ndarray:
    global LAST_RESULTS
    features = np.ascontiguousarray(np.asarray(inputs["features"], dtype=np.float32))
    class_idx = int(np.asarray(inputs["class_idx"]))
    Hv = int(np.asarray(inputs["H"]))
    Wv = int(np.asarray(inputs["W"]))
    gs = int(np.asarray(inputs["grid_size"]))
    assert (Hv, Wv, gs) == (H, W, G), (Hv, Wv, gs)
    emb_table = np.asarray(inputs["emb_table"], dtype=np.float32)
    w1 = np.ascontiguousarray(np.asarray(inputs["W1"], np.float32)[class_idx]).astype(np.float16)
    b1 = np.asarray(inputs["b1"], np.float32)[class_idx]
    w2 = np.ascontiguousarray(np.asarray(inputs["W2"], np.float32)[class_idx]).astype(np.float16)
    b2 = np.asarray(inputs["b2"], np.float32)[class_idx]
    w3 = np.ascontiguousarray(np.asarray(inputs["W3"], np.float32)[class_idx]).astype(np.float16)
    b3 = np.asarray(inputs["b3"], np.float32)[class_idx]

    # grid embeddings: rows gi*64+gj for gi,gj in {0,4,...,60}
    emb4 = np.ascontiguousarray(
        emb_table[: HW].reshape(H, W, 128)[::4, ::4]
    )  # (16,16,128)
    featg = features[:, :, ::4, ::4]  # (B, 256, 16, 16) strided view

    # misc: b1t (128,2) | b2t (128,1) | b3 replicated (128,1)
    misc = np.concatenate(
        [
            b1.reshape(2, 128).T,
            b2.reshape(128, 1),
            np.full((128, 1), float(b3[0]), np.float32),
        ],
        axis=1,
    ).astype(np.float32)

    in_maps = []
    for c in range(8):
        bb, hh = c // 2, c % 2
        # halo rows: local t=0 is north halo, t=1..8 own, t=9 south halo
        gus = [8 * hh - 1] + list(range(8 * hh, 8 * hh + 8)) + [8 * hh + 8]
        feat_core = np.zeros((D, 10, G), np.float16)
        emb_core = np.zeros((10 * G, 128), np.float32)
        for i, gu in enumerate(gus):
            if 0 <= gu < G:
                feat_core[:, i, :] = featg[bb, :, gu, :]
                emb_core[i * G : (i + 1) * G, :] = emb4[gu]
        embt_core = np.ascontiguousarray((0.5 * emb_core).T).astype(np.float16)
        mns = np.ones((1, 256), np.float32)
        # (g,t)-major: t=0 rows sit at indices g*8+0, t=7 at g*8+7
        if hh == 0:
            mns[0, 0:128:8] = 0.0  # maskn: no north neighbor for gi=0
        else:
            mns[0, 135::8] = 0.0  # masks: no south neighbor for gi=15
        in_maps.append(
            {
                "feat": feat_core,
                "embt": embt_core,
                "w1": w1,
                "w2": w2,
                "w3": w3,
                "misc": misc,
                "mns": mns,
            }
        )

    nc = _get_nc()
    res = run_bass_kernel_spmd(nc, in_maps, core_ids=list(range(8)))
    LAST_RESULTS = res

    out = np.empty((B, HW, HW), np.float32)
    for c in range(8):
        bb, hh = c // 2, c % 2
        shard = res.results[c]["a"]  # fp16 (2048, 4096)
        if hh:
            shard = np.roll(shard, 2048, axis=1)
        out[bb, 2048 * hh : 2048 * (hh + 1), :] = shard  # casts fp16 -> f32
    return out


# revision 17
# speedup vs baseline: 1.1128x; 1.1128x over previous
"""Trainium2 Bass kernel for nn_ClassConditionalAffinity.

Problem (hardcoded shapes): B=4, D=256, H=W=64, grid=16 -> HW=4096.
Valid pairs are the 4-neighbors of the 16x16 grid of pixels (0,4,...,60)^2
(manhattan distance 4 <= 5), giving 960 directed pairs per batch. The
output A is (B, 4096, 4096): identity everywhere except the 256 grid rows,
which carry up to 4 sigmoid(MLP) affinities at columns row+-4 / row+-256,
then every row is normalized by its sum.

Sharding: 8 cores = 4 batches x 2 row-halves (2048 rows each). Every core
runs the SAME program; per-core differences are carried by the data:
  - features/embeddings are passed as a 10-grid-row halo window (zero
    padded at the outer boundary) and boundary masks zero the missing
    north/south neighbor values,
  - the upper-half cores write columns rotated by -2048 (mod 4096); the
    host un-rotates with np.roll. Every DMA offset is a compile-time
    constant shared by all 8 cores.

The kernel is HBM-write-bound: 2048x4096 fp16 = 16.8 MB/core (fp16 halves
the f32 traffic; harness tolerance is 2e-2, fp16 adds ~5e-4; the host
casts back to f32 on gather). Output strategy (sync HWDGE ring, in FIFO
order -- each SDMA engine owns fixed partitions and drains its ring in
order, so later DMAs to the same bytes land later):
  1. ONE 16.8 MB full-shard zero DMA whose source is a tiny [128, 128]
     zero tile read through a stride-0 broadcast access pattern.
  2. ONE merged DMA overwriting the 8 odd-block identity diagonals (a
     [128,128] fp16 identity broadcast across blocks; the flat-DRAM
     k-stride 256*4096+256 lands each copy on the block diagonal).
  3. After the MLP: 3 merged DMAs (plus 4 small ones for the wrapping
     block 0) overwriting only the NONZERO patch columns of the 8 even
     blocks: the -256 diagonal, the 136-wide -4/0/+4 band, the +256
     diagonal. Zero flanks are already covered by (1).
The MLP (fp16 weights/activations, f32 PSUM + f32 sums/reciprocal) runs
entirely under the zero-write drain. The tiny V-table scatter goes via
gpsimd (SWDGE) so it never queues behind bulk HWDGE traffic; input loads
go on the scalar HWDGE ring.
"""

import os
import numpy as np

import concourse.bass as bass
import concourse.mybir as mybir
import concourse.tile as tile
from concourse import bacc
from concourse.bass import broadcast_tensor_aps
from concourse.bass_utils import run_bass_kernel_spmd
from concourse.masks import make_identity

F32 = mybir.dt.float32
F16 = mybir.dt.float16
AF = mybir.ActivationFunctionType

B, D, H, W = 4, 256, 64, 64
HW = H * W                      # 4096
G = 16                          # grid points per axis
TG = 8                          # own grid rows (gi) per core
ROWS = 2048                     # rows per core shard
NB = 16                         # 128-row blocks per shard
NPAIR = 496                     # E/W: 8*15 each, N/S: 8*16 each
MPAD = 512
MLP_IN, H1, H2 = 640, 256, 128
KSTR = 256 * HW + 256           # flat stride between consecutive diag/patch blocks

LAST_RESULTS = None             # test.py reads exec_time_ns from here


def _build_nc():
    nc = bacc.Bacc("TRN2", target_bir_lowering=False)

    feat = nc.dram_tensor("feat", [D, 10, G], F16, kind="ExternalInput")
    embt = nc.dram_tensor("embt", [128, 10 * G], F16, kind="ExternalInput")
    w1 = nc.dram_tensor("w1", [128, 5 * H1], F16, kind="ExternalInput")
    w2 = nc.dram_tensor("w2", [128, 2 * H2], F16, kind="ExternalInput")
    w3 = nc.dram_tensor("w3", [H2, 1], F16, kind="ExternalInput")
    misc = nc.dram_tensor("misc", [128, 4], F32, kind="ExternalInput")
    mns = nc.dram_tensor("mns", [1, 256], F32, kind="ExternalInput")
    a = nc.dram_tensor("a", [ROWS, HW], F16, kind="ExternalOutput")

    from contextlib import ExitStack

    with tile.TileContext(nc) as tc, ExitStack() as ctx:
        consts = ctx.enter_context(tc.tile_pool(name="consts", bufs=1))
        dpool = ctx.enter_context(tc.tile_pool(name="dpool", bufs=2))
        psum = ctx.enter_context(tc.tile_pool(name="psum", bufs=1, space="PSUM"))

        aflat = a[:].rearrange("r c -> (r c)")

        def dram_ap(offset, dims):
            return bass.AP(aflat.tensor, offset, dims)

        # ---- tiny constants ----
        # zsrc memset is split DVE/gpsimd so the first bulk write can start
        # ~2 us earlier than a single-engine memset would allow
        zsrc = consts.tile([128, 2048], F16)
        nc.vector.memset(zsrc[:, 0:1024], 0.0)
        nc.gpsimd.memset(zsrc[:, 1024:2048], 0.0)
        identh = consts.tile([128, 128], F16)
        make_identity(nc, identh)
        # zdiag: 256-wide zeros with the identity at cols [64:192) -- the
        # widened diag overwrite keeps every descriptor line at 512 B, above
        # the sub-512B HBM read-modify-write knee
        zdiag = consts.tile([128, 256], F16)
        nc.vector.memset(zdiag, 0.0)
        nc.vector.tensor_copy(zdiag[:, 64:192], identh[:])

        def zeros_to(offset, cols):
            # one zero-write DMA; chunk the stride-0 source so lanes stay
            # uniform (~<=1 MB per DMA, lines >= 512 B)
            if cols <= 2048:
                nc.sync.dma_start(
                    out=dram_ap(offset, [[HW, 128], [1, cols]]), in_=zsrc[:, 0:cols]
                )
            else:
                half = cols // 2
                bi, bo = broadcast_tensor_aps(
                    zsrc[:, 0:half].rearrange("p (j c) -> p j c", j=1),
                    dram_ap(offset, [[HW, 128], [half, 2], [1, half]]),
                )
                nc.sync.dma_start(out=bo, in_=bi)

        # ---- (1) odd blocks: 8 uniform full-width 1 MB zero DMAs ----
        for lb in range(1, NB, 2):
            zeros_to(128 * lb * HW, HW)
        # ---- (2) their identity diagonals: one merged DMA, same ring so
        # its descriptors land after the zeros on every engine ----
        out_diag = dram_ap(128 * HW + 64, [[HW, 128], [KSTR, TG], [1, 256]])
        in_diag = zdiag[:].rearrange("p (j c) -> p j c", j=1)
        bi, bo = broadcast_tensor_aps(in_diag, out_diag)
        nc.sync.dma_start(out=bo, in_=bi)

        # ---- (3) even-block zero stripes around the 640-wide patch
        # windows (disjoint from the patches -> no ordering constraint) ----
        zeros_to(384, 3456)
        for k in range(1, TG):
            r0 = 256 * k
            c0 = 256 * k - 256
            if c0 > 0:
                zeros_to(r0 * HW, c0)
            zeros_to(r0 * HW + c0 + 640, HW - c0 - 640)

        # ---- inputs (scalar HWDGE ring) ----
        g0 = consts.tile([128, 10, G], F16)
        g1 = consts.tile([128, 10, G], F16)
        nc.scalar.dma_start(out=g0, in_=feat[0:128])
        nc.scalar.dma_start(out=g1, in_=feat[128:256])
        emb = consts.tile([128, 10, G], F16)
        nc.scalar.dma_start(out=emb.rearrange("p t g -> p (t g)"), in_=embt[:])
        w1sb = consts.tile([128, 5, H1], F16)
        nc.scalar.dma_start(out=w1sb.rearrange("p k n -> p (k n)"), in_=w1[:])
        w2sb = consts.tile([128, 2, H2], F16)
        nc.scalar.dma_start(out=w2sb.rearrange("p k n -> p (k n)"), in_=w2[:])
        w3sb = consts.tile([128, 1], F16)
        nc.scalar.dma_start(out=w3sb, in_=w3[:])
        miscs = consts.tile([128, 4], F32)
        nc.scalar.dma_start(out=miscs, in_=misc[:])
        mnssb = consts.tile([1, 256], F32)
        nc.scalar.dma_start(out=mnssb, in_=mns[:])
        mn = mnssb[:, 0:128]
        ms = mnssb[:, 128:256]

        # ---- assemble xT (640 x 512) fp16, pair order: E | W | N | S ----
        # pair classes, local own gi index t=0..7 lives at halo row t+1
        xt = [consts.tile([128, MPAD], F16, name=f"xt{k}") for k in range(5)]
        for k in range(5):
            nc.vector.memset(xt[k][:, NPAIR:MPAD], 0.0)

        # pair storage is (g, t)-major: idx = g*8 + t (t contiguous)
        def cview(apx, lo, n):
            return apx[:, lo : lo + n].rearrange("p (g t) -> p g t", t=TG)

        def gswap(apx):
            return apx.rearrange("p t g -> p g t")

        for ki, gt in ((0, g0), (1, g1)):
            f1a, f2a = xt[ki], xt[ki + 2]
            # E: f1=(t,0:15) f2=(t,1:16)
            nc.vector.tensor_copy(cview(f1a, 0, 120), gswap(gt[:, 1:9, 0:15]))
            nc.vector.tensor_copy(cview(f2a, 0, 120), gswap(gt[:, 1:9, 1:16]))
            # W: f1=(t,1:16) f2=(t,0:15)
            nc.vector.tensor_copy(cview(f1a, 120, 120), gswap(gt[:, 1:9, 1:16]))
            nc.vector.tensor_copy(cview(f2a, 120, 120), gswap(gt[:, 1:9, 0:15]))
            # N: f1=own rows, f2=rows above (halo index t)
            nc.vector.tensor_copy(cview(f1a, 240, 128), gswap(gt[:, 1:9, :]))
            nc.vector.tensor_copy(cview(f2a, 240, 128), gswap(gt[:, 0:8, :]))
            # S: f2=rows below (halo index t+2)
            nc.vector.tensor_copy(cview(f1a, 368, 128), gswap(gt[:, 1:9, :]))
            nc.vector.tensor_copy(cview(f2a, 368, 128), gswap(gt[:, 2:10, :]))
        # coord rows: 0.5*(emb[p1]+emb[p2]) with the 0.5 folded in on host
        ct = xt[4]
        nc.vector.tensor_add(cview(ct, 0, 120), gswap(emb[:, 1:9, 0:15]), gswap(emb[:, 1:9, 1:16]))
        nc.vector.tensor_add(cview(ct, 120, 120), gswap(emb[:, 1:9, 1:16]), gswap(emb[:, 1:9, 0:15]))
        nc.vector.tensor_add(cview(ct, 240, 128), gswap(emb[:, 1:9, :]), gswap(emb[:, 0:8, :]))
        nc.vector.tensor_add(cview(ct, 368, 128), gswap(emb[:, 1:9, :]), gswap(emb[:, 2:10, :]))

        # ---- MLP (fp16 in, f32 PSUM, transposed activations) ----
        h1sb = consts.tile([128, 2, MPAD], F16)
        for n in range(2):
            ps1 = psum.tile([128, MPAD], F32)
            for k in range(5):
                nc.tensor.matmul(
                    ps1,
                    w1sb[:, k, 128 * n : 128 * (n + 1)],
                    xt[k][:],
                    start=(k == 0),
                    stop=(k == 4),
                )
            nc.scalar.activation(h1sb[:, n, :], ps1, AF.Relu, bias=miscs[:, n : n + 1])
        ps2 = psum.tile([128, MPAD], F32)
        for k in range(2):
            nc.tensor.matmul(ps2, w2sb[:, k, :], h1sb[:, k, :], start=(k == 0), stop=(k == 1))
        h2sb = consts.tile([128, MPAD], F16)
        nc.scalar.activation(h2sb, ps2, AF.Relu, bias=miscs[:, 2:3])
        ps3 = psum.tile([1, MPAD], F32)
        nc.tensor.matmul(ps3, w3sb[:], h2sb[:], start=True, stop=True)
        vals = consts.tile([1, MPAD], F32)
        nc.scalar.activation(vals, ps3, AF.Sigmoid, bias=miscs[0:1, 3:4])

        # ---- row sums (f32), reciprocal, scaled values -> vall fp16 ----
        vnm = consts.tile([1, 128], F32)
        vsm = consts.tile([1, 128], F32)
        nc.vector.tensor_mul(vnm, vals[:, 240:368], mn)
        nc.vector.tensor_mul(vsm, vals[:, 368:496], ms)

        s = consts.tile([1, 128], F32)
        nc.vector.memset(s, 1.0)
        s3 = s.rearrange("o (g t) -> o g t", t=TG)
        nc.vector.tensor_add(s3[:, 0:15, :], s3[:, 0:15, :], cview(vals, 0, 120))
        nc.vector.tensor_add(s3[:, 1:16, :], s3[:, 1:16, :], cview(vals, 120, 120))
        nc.vector.tensor_add(s, s, vnm[:])
        nc.vector.tensor_add(s, s, vsm[:])
        recip = consts.tile([1, 128], F32)
        nc.vector.reciprocal(recip, s)
        r3 = recip.rearrange("o (g t) -> o g t", t=TG)

        # vall layout (g, k, t); offsets k: 0:-256(N) 1:-4(W) 2:diag 3:+4(E) 4:+256(S)
        vall = consts.tile([1, 16 * 5 * TG], F16)
        nc.vector.memset(vall, 0.0)
        va4 = vall.rearrange("o (g k t) -> o g k t", k=5, t=TG)
        nc.vector.tensor_copy(va4[:, :, 2, :], r3)
        nc.vector.tensor_mul(va4[:, :, 0, :], vnm.rearrange("o (g t) -> o g t", t=TG), r3)
        nc.vector.tensor_mul(va4[:, :, 4, :], vsm.rearrange("o (g t) -> o g t", t=TG), r3)
        nc.vector.tensor_mul(va4[:, 0:15, 3, :], cview(vals, 0, 120), r3[:, 0:15, :])
        nc.vector.tensor_mul(va4[:, 1:16, 1, :], cview(vals, 120, 120), r3[:, 1:16, :])

        # ---- V table (fp16): SWDGE scatter, partition 4g gets 40 values ----
        v = consts.tile([128, 5, TG], F16)
        nc.gpsimd.memset(v, 0.0)
        nc.gpsimd.memset(v[:, 2, :], 1.0)
        with nc.allow_non_contiguous_dma(reason="tiny per-partition scatter"):
            nc.gpsimd.dma_start(
                out=v[0:61:4, :, :],
                in_=vall.rearrange("o (g f) -> o g f", g=16),
            )

        # ---- batched patch build (all 8 blocks at once, fp16, 640-wide:
        # 1280 B lines stay above the 512 B HBM read-modify-write knee) ----
        def vb(k):  # v[:, k, :] broadcast over the diag columns
            return v[:, k, :].rearrange("p (t c) -> p t c", c=1)

        def idb():  # identity broadcast over the 8 blocks
            return identh[:].rearrange("p (j c) -> p j c", j=1)

        ph4 = consts.tile([128, TG, 640], F16)
        nc.vector.memset(ph4[:, :, 128:512], 0.0)
        bi0, bi1 = broadcast_tensor_aps(idb(), vb(0))
        nc.vector.tensor_mul(ph4[:, :, 0:128], bi0, bi1)
        bi0, bi1 = broadcast_tensor_aps(idb(), vb(4))
        nc.vector.tensor_mul(ph4[:, :, 512:640], bi0, bi1)
        bi0, bi1 = broadcast_tensor_aps(idb(), vb(1))
        nc.vector.tensor_mul(ph4[:, :, 252:380], bi0, bi1)
        dt1 = dpool.tile([128, TG, 128], F16)
        bi0, bi1 = broadcast_tensor_aps(idb(), vb(2))
        nc.vector.tensor_mul(dt1, bi0, bi1)
        nc.vector.tensor_add(ph4[:, :, 256:384], ph4[:, :, 256:384], dt1[:])
        dt2 = dpool.tile([128, TG, 128], F16)
        bi0, bi1 = broadcast_tensor_aps(idb(), vb(3))
        nc.vector.tensor_mul(dt2, bi0, bi1)
        nc.vector.tensor_add(ph4[:, :, 260:388], ph4[:, :, 260:388], dt2[:])

        # ---- patch windows (scalar ring; disjoint from all zero writes,
        # so they drain as soon as ph4 is ready) ----
        # wrap block lb=0: window starts at col -256 (mod 4096)
        nc.scalar.dma_start(out=a[0:128, 3840:4096], in_=ph4[:, 0, 0:256])
        nc.scalar.dma_start(out=a[0:128, 0:384], in_=ph4[:, 0, 256:640])
        # blocks lb=2k, k=1..7: window at col 256k-256
        nc.scalar.dma_start(
            out=dram_ap(256 * HW, [[HW, 128], [KSTR, 7], [1, 640]]),
            in_=ph4[:, 1:8, :],
        )
    nc.compile()  # bacc register allocation — required before NEFF compile
    return nc


_NC_CACHE = None


def _get_nc():
    global _NC_CACHE
    if _NC_CACHE is None:
        _NC_CACHE = _build_nc()
    return _NC_CACHE


def kernel(**inputs) -> np.ndarray:
    global LAST_RESULTS
    features = np.ascontiguousarray(np.asarray(inputs["features"], dtype=np.float32))
    class_idx = int(np.asarray(inputs["class_idx"]))
    Hv = int(np.asarray(inputs["H"]))
    Wv = int(np.asarray(inputs["W"]))
    gs = int(np.asarray(inputs["grid_size"]))
    assert (Hv, Wv, gs) == (H, W, G), (Hv, Wv, gs)
    emb_table = np.asarray(inputs["emb_table"], dtype=np.float32)
    w1f = np.asarray(inputs["W1"], np.float32)[class_idx]  # (640, 256)
    w1 = np.ascontiguousarray(
        w1f.reshape(5, 128, H1).transpose(1, 0, 2).reshape(128, 5 * H1)
    ).astype(np.float16)
    b1 = np.asarray(inputs["b1"], np.float32)[class_idx]
    w2f = np.asarray(inputs["W2"], np.float32)[class_idx]  # (256, 128)
    w2 = np.ascontiguousarray(
        w2f.reshape(2, 128, H2).transpose(1, 0, 2).reshape(128, 2 * H2)
    ).astype(np.float16)
    b2 = np.asarray(inputs["b2"], np.float32)[class_idx]
    w3 = np.ascontiguousarray(np.asarray(inputs["W3"], np.float32)[class_idx]).astype(np.float16)
    b3 = np.asarray(inputs["b3"], np.float32)[class_idx]

    # grid embeddings: rows gi*64+gj for gi,gj in {0,4,...,60}
    emb4 = np.ascontiguousarray(
        emb_table[: HW].reshape(H, W, 128)[::4, ::4]
    )  # (16,16,128)
    featg = features[:, :, ::4, ::4]  # (B, 256, 16, 16) strided view

    # misc: b1t (128,2) | b2t (128,1) | b3 replicated (128,1)
    misc = np.concatenate(
        [
            b1.reshape(2, 128).T,
            b2.reshape(128, 1),
            np.full((128, 1), float(b3[0]), np.float32),
        ],
        axis=1,
    ).astype(np.float32)

    in_maps = []
    for c in range(8):
        bb, hh = c // 2, c % 2
        # halo rows: local t=0 is north halo, t=1..8 own, t=9 south halo
        gus = [8 * hh - 1] + list(range(8 * hh, 8 * hh + 8)) + [8 * hh + 8]
        feat_core = np.zeros((D, 10, G), np.float16)
        emb_core = np.zeros((10 * G, 128), np.float32)
        for i, gu in enumerate(gus):
            if 0 <= gu < G:
                feat_core[:, i, :] = featg[bb, :, gu, :]
                emb_core[i * G : (i + 1) * G, :] = emb4[gu]
        embt_core = np.ascontiguousarray((0.5 * emb_core).T).astype(np.float16)
        mns = np.ones((1, 256), np.float32)
        # (g,t)-major: t=0 rows sit at indices g*8+0, t=7 at g*8+7
        if hh == 0:
            mns[0, 0:128:8] = 0.0  # maskn: no north neighbor for gi=0
        else:
            mns[0, 135::8] = 0.0  # masks: no south neighbor for gi=15
        in_maps.append(
            {
                "feat": feat_core,
                "embt": embt_core,
                "w1": w1,
                "w2": w2,
                "w3": w3,
                "misc": misc,
                "mns": mns,
            }
        )

    nc = _get_nc()
    res = run_bass_kernel_spmd(nc, in_maps, core_ids=list(range(8)))
    LAST_RESULTS = res

    out = np.empty((B, HW, HW), np.float32)
    for c in range(8):
        bb, hh = c // 2, c % 2
        shard = res.results[c]["a"]  # fp16 (2048, 4096)
        if hh:
            shard = np.roll(shard, 2048, axis=1)
        out[bb, 2048 * hh : 2048 * (hh + 1), :] = shard  # casts fp16 -> f32
    return out
